# revision 1
# baseline (speedup 1.0000x reference)
"""Trainium2 Bass kernel for nn_Encoder_61770219651232 (dual-quaternion skinning).

Computation per node n (N = 2,000,000):
    qs = W[n, :10] @ qm4            (qm4 = x.reshape(10, 4), shared)
    q  = qs / |qs|                  (normalize)
    y3 = R(q) @ v                   (rotate v = VR[4n:4n+3])
    y  = [y3, r]                    (r = VR[4n+3] passes through)

Strategy (pure data parallel over nodes, 8 cores, all fp32):
  - W loads contiguously as (128, 1920) tiles; each 120-float column chunk
    holds 12 nodes x 10 weights (a "dozen").
  - PE transpose #1: (128, 120) slice -> (120, 128): puts the (node-in-dozen,
    weight) flat offset on partitions, dozens on the free axis.
  - Fused blend+transpose matmul: qt_c = Wt_slice.T @ blockdiag(qm4):
    stationary = a (120, 128) Wt slice, moving = the (120, 48) block-diagonal
    qm4. One matmul both applies qm4 and lands quaternions NODE-MAJOR
    interleaved (128 partitions x [qx qy qz qw] runs) -- exactly matching a
    naturally-loaded VR tile, so no further data movement is needed.
  - DVE/ACT rotation with unnormalized q (no sqrt):
        y3 = v + (2/|q|^2) * (qw*(qv x v) + qv x (qv x v))
    ACT does squares (scale=1/sqrt(2) folds the 2) and PSUM->SBUF copies;
    DVE does the cross products (scalar_tensor_tensor FMAs) and
    reciprocal_approx_fast (2/|q|^2 without sqrt, ~51 ULP).
  - y written in-place into the VR tile, contiguous DMA out.
Scale-relative error vs the fp32 jax reference: ~5e-6.
Cost-model (TimelineSim) estimate: ~102 us/core; DVE-bound (~94% DVE
occupancy; cross-product stages packed into fat 3-component tiles; W loads
issued on the scalar-engine HWDGE ring, VR/y on sync, to split DMA issue).
"""
import sys

sys.path.insert(0, "/opt/trn_rl_repo")

import numpy as np

N_NODES = 2_000_000
N_CORES = 8
MB_NODES = 24576          # nodes per megablock (2048 dozens)
NMB = 11                  # megablocks per core
NPC = MB_NODES * NMB      # 270336 nodes per core
N_PAD = NPC * N_CORES     # 2162688 padded total
GRANULES = [(0, 1), (1, 2), (3, 3), (6, 3), (9, 2)]  # (first mb, num mbs) rotate granules

# "f32" = exact fp32 matmuls (4 cyc/row); "f32r" = single-pass PE mode
# (1-1.5 cyc/row); precision measured empirically in test.py.
MM_MODE = "f32"

_compiled = None


def _build_kernel():
    import concourse.bacc as bacc
    import concourse.tile as tile
    from concourse import mybir

    f32 = mybir.dt.float32
    Alu = mybir.AluOpType
    Act = mybir.ActivationFunctionType

    nc = bacc.Bacc("TRN2", target_bir_lowering=False, debug=False,
                   num_devices=N_CORES)

    w_dram = nc.dram_tensor("w", [NPC * 10], f32, kind="ExternalInput")
    vr_dram = nc.dram_tensor("vr", [NPC * 4], f32, kind="ExternalInput")
    bd_dram = nc.dram_tensor("bd", [120, 48], f32, kind="ExternalInput")
    id_dram = nc.dram_tensor("ident", [128, 128], f32, kind="ExternalInput")
    y_dram = nc.dram_tensor("y", [NPC * 4], f32, kind="ExternalOutput")

    w3 = w_dram.ap().rearrange("(m p e) -> m p e", m=NMB, p=128)      # e=1920
    vr3 = vr_dram.ap().rearrange("(m f e) -> m f e", m=NMB, f=128)    # e=768
    y3 = y_dram.ap().rearrange("(m f e) -> m f e", m=NMB, f=128)

    from contextlib import ExitStack

    with tile.TileContext(nc) as tc, ExitStack() as ctx:
        consts = ctx.enter_context(tc.tile_pool(name="consts", bufs=1))
        wpool = ctx.enter_context(tc.tile_pool(name="wpool", bufs=3))
        wtpool = ctx.enter_context(tc.tile_pool(name="wtpool", bufs=2))
        gran_pool = ctx.enter_context(tc.tile_pool(name="gran", bufs=3))
        scratch = ctx.enter_context(tc.tile_pool(name="scratch", bufs=2))
        wt_psp = ctx.enter_context(tc.tile_pool(name="wt_ps", bufs=2, space="PSUM"))
        qt_psp = ctx.enter_context(tc.tile_pool(name="qt_ps", bufs=2, space="PSUM"))

        bd_sb = consts.tile([120, 48], f32)
        nc.sync.dma_start(out=bd_sb[:], in_=bd_dram.ap())
        id_sb = consts.tile([128, 128], f32)
        nc.sync.dma_start(out=id_sb[:], in_=id_dram.ap())

        def mmv(ap):
            """matmul-operand view, optionally bitcast to float32r"""
            return ap.bitcast(mybir.dt.float32r) if MM_MODE == "f32r" else ap

        for g0, gn in GRANULES:
            fd = 768 * gn            # interleaved free size for this granule
            n_el = fd // 4           # per-component element count
            qt_gran = gran_pool.tile([128, fd], f32, tag="qt_gran")
            vr_gran = gran_pool.tile([128, fd], f32, tag="vr_gran")

            for k in range(gn):
                mb = g0 + k
                # ---- load W megablock + VR slice ----
                w_big = wpool.tile([128, 1920], f32, tag="w_big")
                nc.scalar.dma_start(out=w_big[:], in_=w3[mb])
                nc.sync.dma_start(out=vr_gran[:, 768 * k:768 * (k + 1)],
                                  in_=vr3[mb])
                # ---- T1: 16 PE transposes -> wt_sb (120, 2048) ----
                wt_sb = wtpool.tile([120, 2048], f32, tag="wt_sb")
                for b in range(4):
                    wt_ps = wt_psp.tile([120, 512], f32, tag="wt_ps")
                    for t4 in range(4):
                        t = 4 * b + t4
                        nc.tensor.transpose(
                            mmv(wt_ps[:, 128 * t4:128 * (t4 + 1)]),
                            mmv(w_big[:, 120 * t:120 * (t + 1)]),
                            mmv(id_sb[:]),
                        )
                    nc.scalar.copy(out=wt_sb[:, 512 * b:512 * (b + 1)],
                                   in_=wt_ps[:])
                # ---- fused blend+transpose: qt_c = Wt_slice.T @ BD ----
                # out[f, 4s+j] = sum_k Wt[k, 128c+f] * BD[k, 4s+j]
                #             = qs_j(node 12*(16f+c)+s): node-major interleaved
                for bank in range(2):
                    qt_ps = qt_psp.tile([128, 384], f32, tag="qt_ps")
                    for cc in range(8):
                        c = 8 * bank + cc
                        nc.tensor.matmul(
                            qt_ps[:, 48 * cc:48 * (cc + 1)],
                            mmv(wt_sb[:, 128 * c:128 * (c + 1)]),
                            mmv(bd_sb[:]),
                        )
                    off = 768 * k + 384 * bank
                    nc.scalar.copy(out=qt_gran[:, off:off + 384], in_=qt_ps[:])

            # ---- rotate on the whole granule ----
            Q = qt_gran[:, :fd].rearrange("p (n e) -> p n e", e=4)
            V = vr_gran[:, :fd].rearrange("p (n e) -> p n e", e=4)
            qx, qy, qz, qw = (Q[:, :, i:i + 1] for i in range(4))
            vx, vy, vz = (V[:, :, i:i + 1] for i in range(3))

            def st(tag, width=1):
                return scratch.tile([128, n_el, width], f32, tag=tag, name=tag)

            # |q|^2/2 via ACT squares with scale 1/sqrt(2), tree-added fat
            isq = float(np.sqrt(0.5))
            sqp = st("sqp", 4)
            for i, qc in enumerate((qx, qy, qz, qw)):
                nc.scalar.activation(sqp[:, :, i:i + 1], qc, Act.Square, scale=isq)
            s2 = st("s2", 2)
            nc.vector.tensor_add(s2[:], sqp[:, :, 0:2], sqp[:, :, 2:4])
            n2h = st("n2h")
            nc.vector.tensor_add(n2h[:], s2[:, :, 0:1], s2[:, :, 1:2])
            gg = st("gg")
            nc.vector.reciprocal_approx_fast(out=gg[:], in_=n2h[:])  # 2/|q|^2

            # t = qv x v (into fat tile T): fat products then one fat sub
            T = st("T", 3)
            C = st("C", 3)
            P = st("P", 3)
            Qm = st("Qm", 3)
            for (i, (a1, b1), (a2, b2)) in (
                (0, (qy, vz), (qz, vy)),
                (1, (qz, vx), (qx, vz)),
                (2, (qx, vy), (qy, vx)),
            ):
                nc.vector.tensor_mul(P[:, :, i:i + 1], a1, b1)
                nc.vector.tensor_mul(Qm[:, :, i:i + 1], a2, b2)
            nc.vector.scalar_tensor_tensor(
                out=T[:], in0=Qm[:], scalar=-1.0, in1=P[:],
                op0=Alu.mult, op1=Alu.add)

            # c = qv x t (into C), wt = qw*t (into WT)
            WT = st("WT", 3)
            tv = [T[:, :, i:i + 1] for i in range(3)]
            for i in range(3):
                nc.vector.tensor_mul(WT[:, :, i:i + 1], qw, tv[i])
            for (i, (a1, b1), (a2, b2)) in (
                (0, (qy, tv[2]), (qz, tv[1])),
                (1, (qz, tv[0]), (qx, tv[2])),
                (2, (qx, tv[1]), (qy, tv[0])),
            ):
                nc.vector.tensor_mul(P[:, :, i:i + 1], a1, b1)
                nc.vector.tensor_mul(Qm[:, :, i:i + 1], a2, b2)
            nc.vector.scalar_tensor_tensor(
                out=C[:], in0=Qm[:], scalar=-1.0, in1=P[:],
                op0=Alu.mult, op1=Alu.add)

            # m = c + wt (fat); e = m*g; y = v + e (fat). For the final
            # granule, per-mb slices let the last y-store DMAs overlap the
            # tail of the rotate instead of waiting for all of it.
            tail_splits = ((0, n_el),) if g0 + gn < NMB else tuple(
                (192 * kk2, 192 * (kk2 + 1)) for kk2 in range(gn))
            for lo, hi in tail_splits:
                nc.vector.tensor_add(C[:, lo:hi, :], C[:, lo:hi, :],
                                     WT[:, lo:hi, :])
                for i in range(3):
                    nc.vector.tensor_mul(C[:, lo:hi, i:i + 1],
                                         C[:, lo:hi, i:i + 1], gg[:, lo:hi, :])
                nc.vector.tensor_add(V[:, lo:hi, 0:3], C[:, lo:hi, :],
                                     V[:, lo:hi, 0:3])

            # ---- store y (in-place in vr_gran) ----
            for k in range(gn):
                nc.sync.dma_start(out=y3[g0 + k],
                                  in_=vr_gran[:, 768 * k:768 * (k + 1)])

    nc.compile()
    return nc


def _get_compiled():
    global _compiled
    if _compiled is None:
        _compiled = _build_kernel()
    return _compiled


def kernel(x, weights, VR):
    from concourse import bass_utils

    x = np.asarray(x, dtype=np.float32)
    weights = np.asarray(weights, dtype=np.float32)
    VR = np.asarray(VR, dtype=np.float32)

    qm4 = x.reshape(10, 4)
    bd = np.zeros((120, 48), np.float32)
    for s in range(12):
        bd[10 * s:10 * s + 10, 4 * s:4 * s + 4] = qm4
    ident = np.eye(128, dtype=np.float32)

    w_pad = np.zeros((N_PAD, 10), np.float32)
    w_pad[:N_NODES] = weights
    vr_pad = np.zeros(N_PAD * 4, np.float32)
    vr_pad[:N_NODES * 4] = VR

    w_shards = w_pad.reshape(N_CORES, NPC * 10)
    vr_shards = vr_pad.reshape(N_CORES, NPC * 4)

    nc = _get_compiled()
    in_maps = [
        {"w": w_shards[c], "vr": vr_shards[c], "bd": bd, "ident": ident}
        for c in range(N_CORES)
    ]
    res = bass_utils.run_bass_kernel_spmd(nc, in_maps, core_ids=list(range(N_CORES)))
    y = np.concatenate([res.results[c]["y"] for c in range(N_CORES)])
    return y[:N_NODES * 4].astype(np.float32)


if __name__ == "__main__":
    # quick self-check with random data
    rng = np.random.default_rng(0)
    x = rng.standard_normal(40).astype(np.float32)
    W = (rng.standard_normal((N_NODES, 10)) * 0.1).astype(np.float32)
    VR = rng.standard_normal(N_NODES * 4).astype(np.float32)
    y = kernel(x, weights=W, VR=VR)
    print("kernel ran, y shape", y.shape, y[:8])



# revision 16
# speedup vs baseline: 1.2485x; 1.2485x over previous
"""Trainium2 Bass kernel for nn_Encoder_61770219651232 (dual-quaternion skinning).

Computation per node n (N = 2,000,000):
    qs = W[n, :10] @ qm4            (qm4 = x.reshape(10, 4), shared)
    q  = qs / |qs|                  (normalize)
    y3 = R(q) @ v                   (rotate v = VR[4n:4n+3])
    y  = [y3, r]                    (r = VR[4n+3] passes through)

Strategy (pure data parallel over nodes, 8 cores):
  - W is pre-transposed AND pre-cast to fp16 on the host into the exact
    stationary-operand layout the blend matmuls want: per megablock a
    (120, 2048) tile whose partition axis is (node-in-dozen s, weight k)
    and free axis is (chunk c, out-partition p).  This removes the whole
    on-chip PE-transpose + PSUM->SBUF copy pipeline of the previous
    version and halves W's HBM traffic.
  - Blend: qt = Wt_slice.T @ blockdiag(qm4) in fp16 (PSUM accum fp32):
    lands quaternions node-major interleaved (128 partitions x
    [qx qy qz qw] runs), matching a naturally-loaded VR tile.
  - Rotation with unnormalized q (no sqrt):
        y3 = v + (2/|q|^2) * (qw*(qv x v) + qv x (qv x v))
    split across THREE engines to balance busy time:
      ACT:  squares of q (one fat (n,4) activation, scale 1/sqrt(2))
            + PSUM->SBUF evacuation of qt.
      DVE:  reciprocal_approx_fast (2/|q|^2), first cross product
            t = qv x v, wt = qw*t, final y3 = v + gg*(c+wt).
      Pool: |q|^2 adds, second cross product c = qv x t, m = c + wt
            (gpsimd scalar_tensor_tensor ops).
  - y written in-place into the VR tile, contiguous DMA out.
Cost-model (TimelineSim) estimate: ~50 us/core (vs 102 us for the
transpose-on-chip all-DVE version); DVE/Pool ~47 us busy each, DMA 39 us.
"""
import sys

sys.path.insert(0, "/opt/trn_rl_repo")

import numpy as np

N_NODES = 2_000_000
N_CORES = 8
MB_NODES = 24576          # nodes per megablock (128 partitions x 192 nodes)
NMB = 11                  # megablocks per core
NPC = MB_NODES * NMB      # 270336 nodes per core
N_PAD = NPC * N_CORES     # 2162688 padded total
GRANULES = [(0, 1), (1, 2), (3, 2), (5, 2), (7, 2), (9, 1), (10, 1)]  # (mb0, n)

W_DT = "f16"              # "f16" | "f32": dtype of the staged W + blend matmul

_compiled = None


def _build_kernel():
    import concourse.bacc as bacc
    import concourse.tile as tile
    from concourse import mybir

    f32 = mybir.dt.float32
    wdt = mybir.dt.float16 if W_DT == "f16" else f32
    Alu = mybir.AluOpType
    Act = mybir.ActivationFunctionType

    nc = bacc.Bacc("TRN2", target_bir_lowering=False, debug=False,
                   num_devices=N_CORES)

    wt_dram = nc.dram_tensor("wt", [NMB * 120 * 2048], wdt, kind="ExternalInput")
    vr_dram = nc.dram_tensor("vr", [NPC * 4], f32, kind="ExternalInput")
    bd_dram = nc.dram_tensor("bd", [120, 48], wdt, kind="ExternalInput")
    y_dram = nc.dram_tensor("y", [NPC * 4], f32, kind="ExternalOutput")

    wt3 = wt_dram.ap().rearrange("(m p e) -> m p e", m=NMB, p=120)    # e=2048
    vr3 = vr_dram.ap().rearrange("(m f e) -> m f e", m=NMB, f=128)    # e=768
    y3 = y_dram.ap().rearrange("(m f e) -> m f e", m=NMB, f=128)

    from contextlib import ExitStack

    with tile.TileContext(nc) as tc, ExitStack() as ctx:
        consts = ctx.enter_context(tc.tile_pool(name="consts", bufs=1))
        wtpool = ctx.enter_context(tc.tile_pool(name="wtpool", bufs=5))
        gran_pool = ctx.enter_context(tc.tile_pool(name="gran", bufs=4))
        scratch = ctx.enter_context(tc.tile_pool(name="scratch", bufs=2))
        xpool = ctx.enter_context(tc.tile_pool(name="xpool", bufs=3))
        qt_psp = ctx.enter_context(tc.tile_pool(name="qt_ps", bufs=4, space="PSUM"))

        bd_sb = consts.tile([120, 48], wdt)
        nc.sync.dma_start(out=bd_sb[:], in_=bd_dram.ap())

        prev = None   # granule awaiting its H stage (y3 = v + gg*m)
        prev2 = None  # granule awaiting its y-store DMAs

        def emit_H(C, gg, V, vr_gran, n_el, g0, gn):
            # DVE: y3 = v + gg*m (in-place into the VR tile)
            for kk in range(gn):
                lo, hi = 192 * kk, 192 * (kk + 1)
                for i in range(3):
                    nc.vector.tensor_mul(C[:, lo:hi, i:i + 1],
                                         C[:, lo:hi, i:i + 1], gg[:, lo:hi, :])
                nc.vector.tensor_add(V[:, lo:hi, 0:3], C[:, lo:hi, :],
                                     V[:, lo:hi, 0:3])

        def emit_stores(C, gg, V, vr_gran, n_el, g0, gn):
            # y-store DMAs go on the ACT ring two granules late so their
            # H-stage sem-waits never block the next granule's qt evacs.
            for kk in range(gn):
                nc.scalar.dma_start(out=y3[g0 + kk],
                                    in_=vr_gran[:, 768 * kk:768 * (kk + 1)])

        for g0, gn in GRANULES:
            fd = 768 * gn            # interleaved free size for this granule
            n_el = fd // 4           # per-component element count
            qt_gran = gran_pool.tile([128, fd], f32, tag="qt_gran")
            vr_gran = gran_pool.tile([128, fd], f32, tag="vr_gran")

            for k in range(gn):
                mb = g0 + k
                # ---- load pre-transposed W stationary tile + VR slice ----
                wt_sb = wtpool.tile([120, 2048], wdt, tag="wt_sb")
                nc.sync.dma_start(out=wt_sb[:], in_=wt3[mb])
                nc.sync.dma_start(out=vr_gran[:, 768 * k:768 * (k + 1)],
                                  in_=vr3[mb])
                # ---- blend: qt_c = Wt_slice.T @ BD ----
                # out[f, 4s+j] = sum_k Wt[(s,k), 128c+f] * BD[(s,k), 4s+j]
                #             -> qs_j(node 192*f + 12c + s): node-major runs
                for bank in range(2):
                    qt_ps = qt_psp.tile([128, 384], f32, tag="qt_ps")
                    for cc in range(8):
                        c = 8 * bank + cc
                        nc.tensor.matmul(
                            qt_ps[:, 48 * cc:48 * (cc + 1)],
                            wt_sb[:, 128 * c:128 * (c + 1)],
                            bd_sb[:],
                        )
                    off = 768 * k + 384 * bank
                    nc.scalar.copy(out=qt_gran[:, off:off + 384], in_=qt_ps[:])

            # ---- rotate on the whole granule ----
            Q = qt_gran[:, :fd].rearrange("p (n e) -> p n e", e=4)
            V = vr_gran[:, :fd].rearrange("p (n e) -> p n e", e=4)
            qx, qy, qz, qw = (Q[:, :, i:i + 1] for i in range(4))
            vx, vy, vz = (V[:, :, i:i + 1] for i in range(3))

            def st(tag, width=1):
                return scratch.tile([128, n_el, width], f32, tag=tag, name=tag)

            def xt(tag, width=1):
                # tiles on the DVE->Pool->DVE chain: ring of 3 so DVE can
                # run up to two granules ahead of Pool's reads (WAR slack)
                return xpool.tile([128, n_el, width], f32, tag=tag, name=tag)

            # |q|^2/2 via one fat ACT square with scale 1/sqrt(2); adds on DVE
            isq = float(np.sqrt(0.5))
            sqp = st("sqp", 4)
            nc.scalar.activation(sqp[:], Q[:, :, 0:4], Act.Square, scale=isq)
            s2 = st("s2", 2)
            nc.vector.tensor_add(s2[:], sqp[:, :, 0:2], sqp[:, :, 2:4])
            n2h = st("n2h")
            nc.vector.tensor_add(n2h[:], s2[:, :, 0:1], s2[:, :, 1:2])
            gg = st("gg")
            nc.vector.reciprocal_approx_fast(out=gg[:], in_=n2h[:])  # 2/|q|^2

            # DVE: t = qv x v (fat products then one fat stt-sub)
            T = xt("T", 3)
            P = xt("P", 3)
            Qm = xt("Qm", 3)
            for (i, (a1, b1), (a2, b2)) in (
                (0, (qy, vz), (qz, vy)),
                (1, (qz, vx), (qx, vz)),
                (2, (qx, vy), (qy, vx)),
            ):
                nc.vector.tensor_mul(P[:, :, i:i + 1], a1, b1)
                nc.vector.tensor_mul(Qm[:, :, i:i + 1], a2, b2)
            nc.vector.scalar_tensor_tensor(
                out=T[:], in0=Qm[:], scalar=-1.0, in1=P[:],
                op0=Alu.mult, op1=Alu.add)

            # DVE: wt = qw*t
            WT = xt("WT", 3)
            tv = [T[:, :, i:i + 1] for i in range(3)]
            for i in range(3):
                nc.vector.tensor_mul(WT[:, :, i:i + 1], qw, tv[i])

            # Pool (gpsimd): c = qv x t, reusing the P/Qm tiles (their
            # D-stage values are dead once T is built)
            C = xt("C", 3)
            for (i, (a1, b1), (a2, b2)) in (
                (0, (qy, tv[2]), (qz, tv[1])),
                (1, (qz, tv[0]), (qx, tv[2])),
                (2, (qx, tv[1]), (qy, tv[0])),
            ):
                nc.gpsimd.tensor_mul(P[:, :, i:i + 1], a1, b1)
                nc.gpsimd.tensor_mul(Qm[:, :, i:i + 1], a2, b2)
            nc.gpsimd.tensor_sub(C[:], P[:], Qm[:])

            # Pool: m = c + wt (into C)
            nc.gpsimd.tensor_add(C[:], C[:], WT[:])

            # Software pipelining: the previous granule's H stage (DVE) is
            # emitted here, so in program order DVE reaches it while Pool
            # chews on THIS granule's cross product instead of stalling;
            # y stores trail one more granule (see emit_stores).
            if prev2 is not None:
                emit_stores(*prev2)
            if prev is not None:
                emit_H(*prev)
            prev2 = prev
            prev = (C, gg, V, vr_gran, n_el, g0, gn)

        if prev2 is not None:
            emit_stores(*prev2)
        emit_H(*prev)
        emit_stores(*prev)

    nc.compile()
    return nc


def _get_compiled():
    global _compiled
    if _compiled is None:
        _compiled = _build_kernel()
    return _compiled


def _host_wt(weights_padded):
    """Pre-transpose W into per-core (NMB, 120, 2048) stationary tiles.

    Node n of a core maps to (mb, p, c, s): n = mb*24576 + p*192 + 12*c + s.
    Stationary tile row (s, k) = 10*s + k, column (c, p) = 128*c + p.
    """
    wdt = np.float16 if W_DT == "f16" else np.float32
    w = weights_padded.reshape(N_CORES, NMB, 128, 16, 12, 10)
    #                  core      mb   p    c   s   k  -> core mb s k c p
    wt = np.ascontiguousarray(w.transpose(0, 1, 4, 5, 3, 2)).astype(wdt)
    return wt.reshape(N_CORES, NMB * 120 * 2048)


def kernel(x, weights, VR):
    from concourse import bass_utils

    x = np.asarray(x, dtype=np.float32)
    weights = np.asarray(weights, dtype=np.float32)
    VR = np.asarray(VR, dtype=np.float32)

    wdt = np.float16 if W_DT == "f16" else np.float32
    qm4 = x.reshape(10, 4)
    bd = np.zeros((120, 48), np.float32)
    for s in range(12):
        bd[10 * s:10 * s + 10, 4 * s:4 * s + 4] = qm4
    bd = bd.astype(wdt)

    w_pad = np.zeros((N_PAD, 10), np.float32)
    w_pad[:N_NODES] = weights
    vr_pad = np.zeros(N_PAD * 4, np.float32)
    vr_pad[:N_NODES * 4] = VR

    wt_shards = _host_wt(w_pad)
    vr_shards = vr_pad.reshape(N_CORES, NPC * 4)

    nc = _get_compiled()
    in_maps = [
        {"wt": wt_shards[c], "vr": vr_shards[c], "bd": bd}
        for c in range(N_CORES)
    ]
    res = bass_utils.run_bass_kernel_spmd(nc, in_maps, core_ids=list(range(N_CORES)))
    y = np.concatenate([res.results[c]["y"] for c in range(N_CORES)])
    return y[:N_NODES * 4].astype(np.float32)


if __name__ == "__main__":
    # quick self-check with random data
    rng = np.random.default_rng(0)
    x = rng.standard_normal(40).astype(np.float32)
    W = (rng.standard_normal((N_NODES, 10)) * 0.1).astype(np.float32)
    VR = rng.standard_normal(N_NODES * 4).astype(np.float32)
    y = kernel(x, weights=W, VR=VR)
    print("kernel ran, y shape", y.shape, y[:8])


# revision 18
# speedup vs baseline: 1.8308x; 1.4663x over previous
"""Trainium2 Bass kernel for nn_Encoder_61770219651232 (dual-quaternion skinning).

Computation per node n (N = 2,000,000):
    qs = W[n, :10] @ qm4            (qm4 = x.reshape(10, 4), shared)
    q  = qs / |qs|                  (normalize)
    y3 = R(q) @ v                   (rotate v = VR[4n:4n+3])
    y  = [y3, r]                    (r = VR[4n+3] passes through)

v3 strategy (pure data parallel over nodes, 8 cores):
  - W pre-transposed + pre-cast to fp16 on the host into the stationary
    layout of the blend matmuls: per megablock a (120, 2048) tile with
    (node-in-dozen s, weight k) on partitions and (chunk c, out-node p)
    on the free axis.
  - Blend matmul columns PERMUTED so each dozen's output lands
    PLANAR-BY-12: [12*qx 12*qy 12*qz 12*qw] per 48-column chunk.  Every
    per-component view then has a packed [1,12] innermost run, which is
    what the DVE's 2x_1p fp16 mode needs (0.52 ns/el instead of 1.04).
  - VR planarized on the HOST into per-granule fp16 [x|y|z]-plane
    blocks (r never goes on chip); y3 stored back in the same planar
    fp16 layout and re-interleaved on the host, r copied from the fp32
    input directly.
  - Rotation, engine split (per-node elements):
      ACT:  qt PSUM->SBUF evac (fp32->fp16, 4 el) + fat q squares (4)
      Pool: |q|^2 halves+sum (3 fp32 adds) + t' = (2/|q|^2)*t (3)
      DVE:  recip_approx(|q|^2/2) (1 el); both cross products, qw*t',
            m = c + wt, y3 = v + m  (27 el, all fp16 packed @ 2x rate)
    t is pre-scaled by 2/|q|^2 BEFORE the second cross product is
    rounded to fp16, so every fp16 intermediate is O(|v|) -- no
    subnormal-flush blow-ups for small-|q| nodes.
  - Software pipelining (emission order = per-engine program order):
    body g emits  loads/blend/evac/sq(g); Pool s2+n2h(g); DVE D(g)
    [t = qv x v] + recip(g); Pool t'(g-1); DVE phase-B(g-1)
    [wt, c, m]; DVE Y(g-2) [y3 = v + m]; ACT store(g-3).
    Each cross-engine hop trails a granule so no engine ever waits on
    a same-granule round trip.
"""
import sys

sys.path.insert(0, "/opt/trn_rl_repo")

import numpy as np

N_NODES = 2_000_000
N_CORES = 8
MB_NODES = 24576          # nodes per megablock (128 partitions x 192 nodes)
NMB = 11                  # megablocks per core
NPC = MB_NODES * NMB      # 270336 nodes per core
N_PAD = NPC * N_CORES     # 2162688 padded total
GRANULES = [(0, 1), (1, 2), (3, 3), (6, 3), (9, 2)]  # (first mb, num mbs)

W_DT = "f16"              # "f16" | "f32": dtype of the staged W + blend matmul

_compiled = None


def _granule_offsets():
    offs, off = [], 0
    for g0, gn in GRANULES:
        offs.append(off)
        off += 128 * 3 * 192 * gn
    return offs, off


def _build_kernel():
    import concourse.bacc as bacc
    import concourse.tile as tile
    from concourse import mybir

    f32 = mybir.dt.float32
    f16 = mybir.dt.float16
    wdt = f16 if W_DT == "f16" else f32
    Act = mybir.ActivationFunctionType

    nc = bacc.Bacc("TRN2", target_bir_lowering=False, debug=False,
                   num_devices=N_CORES)

    y_offs, y_total = _granule_offsets()

    wt_dram = nc.dram_tensor("wt", [NMB * 120 * 2048], wdt, kind="ExternalInput")
    vr_dram = nc.dram_tensor("vr", [y_total], f16, kind="ExternalInput")
    bd_dram = nc.dram_tensor("bd", [120, 48], wdt, kind="ExternalInput")
    y_dram = nc.dram_tensor("y", [y_total], f16, kind="ExternalOutput")

    wt3 = wt_dram.ap().rearrange("(m p e) -> m p e", m=NMB, p=120)    # e=2048
    vr_flat = vr_dram.ap()
    y_flat = y_dram.ap()

    from contextlib import ExitStack

    with tile.TileContext(nc) as tc, ExitStack() as ctx:
        consts = ctx.enter_context(tc.tile_pool(name="consts", bufs=1))
        wtpool = ctx.enter_context(tc.tile_pool(name="wtpool", bufs=5))
        gran_pool = ctx.enter_context(tc.tile_pool(name="gran", bufs=4))
        scratch = ctx.enter_context(tc.tile_pool(name="scratch", bufs=2))
        xpool = ctx.enter_context(tc.tile_pool(name="xpool", bufs=3))
        qt_psp = ctx.enter_context(tc.tile_pool(name="qt_ps", bufs=4, space="PSUM"))

        bd_sb = consts.tile([120, 48], wdt)
        nc.sync.dma_start(out=bd_sb[:], in_=bd_dram.ap())

        def plane_views(t, width):
            return [t[:, i, :].rearrange("p (c s) -> p c s", s=12)
                    for i in range(width)]

        # pipeline registers: granule state dicts at each trailing stage
        pA = None   # awaiting phase-B (wt, c, m)
        pB = None   # awaiting Y (y3 = v + m)
        pC = None   # awaiting y store

        def emit_phaseB(st8):
            qpl, Tpl, n_el = st8["qpl"], st8["Tpl"], st8["n_el"]
            WT = scratch.tile([128, 3, st8["n_el"]], f16, tag="WT")
            WTl = plane_views(WT, 3)
            for i in range(3):
                nc.vector.tensor_mul(WTl[i], qpl[3], Tpl[i])
            P2 = scratch.tile([128, 3, n_el], f16, tag="P2")
            Qm2 = scratch.tile([128, 3, n_el], f16, tag="Qm2")
            P2l, Qm2l = plane_views(P2, 3), plane_views(Qm2, 3)
            for i, (a, b) in enumerate(((1, 2), (2, 0), (0, 1))):
                nc.vector.tensor_mul(P2l[i], qpl[a], Tpl[b])
            for i, (a, b) in enumerate(((2, 1), (0, 2), (1, 0))):
                nc.vector.tensor_mul(Qm2l[i], qpl[a], Tpl[b])
            M = xpool.tile([128, 3, n_el], f16, tag="M")
            nc.vector.tensor_sub(M[:], P2[:], Qm2[:])
            nc.vector.tensor_add(M[:], M[:], WT[:])
            st8["M"] = M

        def emit_Y(st8):
            nc.vector.tensor_add(st8["Vp"][:], st8["M"][:], st8["Vp"][:])

        def emit_store(st8):
            g_idx, n_el = st8["g_idx"], st8["n_el"]
            nc.scalar.dma_start(
                out=y_flat[y_offs[g_idx]:y_offs[g_idx] + 128 * 3 * n_el]
                .rearrange("(p e) -> p e", p=128),
                in_=st8["Vp"][:].rearrange("p c e -> p (c e)"))

        for g_idx, (g0, gn) in enumerate(GRANULES):
            fd = 768 * gn            # qt free size for this granule
            n_el = 192 * gn          # per-component element count
            qt16 = gran_pool.tile([128, fd], f16, tag="qt16")
            Vp = gran_pool.tile([128, 3, n_el], f16, tag="Vp")

            # planar VR block for the whole granule in one DMA
            nc.sync.dma_start(
                out=Vp[:].rearrange("p c e -> p (c e)"),
                in_=vr_flat[y_offs[g_idx]:y_offs[g_idx] + 128 * 3 * n_el]
                .rearrange("(p e) -> p e", p=128))

            for k in range(gn):
                mb = g0 + k
                wt_sb = wtpool.tile([120, 2048], wdt, tag="wt_sb")
                nc.sync.dma_start(out=wt_sb[:], in_=wt3[mb])
                # ---- blend: qt_c = Wt_slice.T @ BD (planar-by-12 cols) ----
                for bank in range(2):
                    qt_ps = qt_psp.tile([128, 384], f32, tag="qt_ps")
                    for cc in range(8):
                        c = 8 * bank + cc
                        nc.tensor.matmul(
                            qt_ps[:, 48 * cc:48 * (cc + 1)],
                            wt_sb[:, 128 * c:128 * (c + 1)],
                            bd_sb[:],
                        )
                    off = 768 * k + 384 * bank
                    nc.scalar.copy(out=qt16[:, off:off + 384], in_=qt_ps[:])

            # planar-by-12 component views of q and v
            Q4 = qt16[:, :fd].rearrange("p (c j s) -> p c j s", j=4, s=12)
            qpl = [Q4[:, :, j, :] for j in range(4)]      # x,y,z,w planes
            vpl = plane_views(Vp, 3)

            # ACT: squares of the whole qt tile in one fat op (fp32 out,
            # scale 1/sqrt(2) so the summed planes give |q|^2/2).  sqp
            # inherits qt16's chunk-interleaved layout.
            isq = float(np.sqrt(0.5))
            sqp = xpool.tile([128, fd], f32, tag="sqp")
            nc.scalar.activation(sqp[:], qt16[:, :fd], Act.Square, scale=isq)
            sq4 = sqp[:, :fd].rearrange("p (c j s) -> p c j s", j=4, s=12)
            sqpl = [sq4[:, :, j, :] for j in range(4)]

            # Pool: |q|^2/2 = (x2+y2) + (z2+w2), fp32
            s2 = xpool.tile([128, 2, n_el], f32, tag="s2")
            s2l = plane_views(s2, 2)
            nc.gpsimd.tensor_add(s2l[0], sqpl[0], sqpl[1])
            nc.gpsimd.tensor_add(s2l[1], sqpl[2], sqpl[3])
            n2h = xpool.tile([128, 1, n_el], f32, tag="n2h")
            nc.gpsimd.tensor_add(
                n2h[:, 0, :].rearrange("p (c s) -> p c s", s=12),
                s2l[0], s2l[1])

            # DVE: t = qv x v (6 packed fp16 muls + 1 fat sub)
            P = scratch.tile([128, 3, n_el], f16, tag="P")
            Qm = scratch.tile([128, 3, n_el], f16, tag="Qm")
            Pl, Qml = plane_views(P, 3), plane_views(Qm, 3)
            T = xpool.tile([128, 3, n_el], f16, tag="T")
            for i, (a, b) in enumerate(((1, 2), (2, 0), (0, 1))):
                nc.vector.tensor_mul(Pl[i], qpl[a], vpl[b])
            for i, (a, b) in enumerate(((2, 1), (0, 2), (1, 0))):
                nc.vector.tensor_mul(Qml[i], qpl[a], vpl[b])
            nc.vector.tensor_sub(T[:], P[:], Qm[:])

            # DVE: gg = 2/|q|^2 (feeds Pool's t' next body)
            gg = xpool.tile([128, 1, n_el], f32, tag="gg")
            nc.vector.reciprocal_approx_fast(out=gg[:, 0, :], in_=n2h[:, 0, :])

            cur = {"g_idx": g_idx, "n_el": n_el, "qpl": qpl, "Vp": Vp,
                   "T": T, "gg": gg}

            # ---- trailing pipeline stages ----
            if pA is not None:
                # Pool: t' = gg * t (prescale keeps fp16 values O(|v|))
                nTp = xpool.tile([128, 3, pA["n_el"]], f16, tag="Tp")
                for i in range(3):
                    nc.gpsimd.tensor_mul(nTp[:, i, :], pA["T"][:, i, :],
                                         pA["gg"][:, 0, :])
                pA["Tpl"] = plane_views(nTp, 3)
                emit_phaseB(pA)
            if pB is not None:
                emit_Y(pB)
            if pC is not None:
                emit_store(pC)
            pC = pB
            pB = pA
            pA = cur

        # drain the pipeline
        for _ in range(3):
            if pA is not None:
                nTp = xpool.tile([128, 3, pA["n_el"]], f16, tag="Tp")
                for i in range(3):
                    nc.gpsimd.tensor_mul(nTp[:, i, :], pA["T"][:, i, :],
                                         pA["gg"][:, 0, :])
                pA["Tpl"] = plane_views(nTp, 3)
                emit_phaseB(pA)
            if pB is not None:
                emit_Y(pB)
            if pC is not None:
                emit_store(pC)
            pC = pB
            pB = pA
            pA = None

    nc.compile()
    return nc


def _get_compiled():
    global _compiled
    if _compiled is None:
        _compiled = _build_kernel()
    return _compiled


def _host_wt(weights_padded):
    """Pre-transpose W into per-core (NMB, 120, 2048) stationary tiles.

    Node n of a core maps to (mb, p, c, s): n = mb*24576 + p*192 + 12*c + s.
    Stationary tile row (s, k) = 10*s + k, column (c, p) = 128*c + p.
    """
    wdt = np.float16 if W_DT == "f16" else np.float32
    w = weights_padded.reshape(N_CORES, NMB, 128, 16, 12, 10)
    wt = np.ascontiguousarray(w.transpose(0, 1, 4, 5, 3, 2)).astype(wdt)
    return wt.reshape(N_CORES, NMB * 120 * 2048)


def _host_vr_planar(vr_padded):
    """VR (N_PAD*4,) fp32 -> per-core planar fp16 granule blocks.

    Per granule: (128, 3, 192*gn) with plane col = 192*k + 12*c + s for
    mb index k in the granule; matches the on-chip Vp tile layout.
    """
    v3 = vr_padded.reshape(N_CORES, NMB, 128, 16, 12, 4)[..., :3]
    v3 = v3.astype(np.float16)
    _, y_total = _granule_offsets()
    out = np.empty((N_CORES, y_total), np.float16)
    for c in range(N_CORES):
        off = 0
        for g0, gn in GRANULES:
            blk = v3[c, g0:g0 + gn]                    # (gn,128,16,12,3)
            blk = blk.transpose(1, 4, 0, 2, 3)          # (128,3,gn,16,12)
            n = blk.size
            out[c, off:off + n] = blk.reshape(-1)
            off += n
    return out


def _host_y_reassemble(y_blocks, vr_full):
    """Per-core planar fp16 y blocks -> full (N_NODES*4,) fp32 output."""
    y = np.empty((N_PAD, 4), np.float32)
    for c in range(N_CORES):
        off = 0
        base = c * NPC
        for g0, gn in GRANULES:
            n = 128 * 3 * 192 * gn
            blk = y_blocks[c][off:off + n].reshape(128, 3, gn, 16, 12)
            blk = blk.transpose(2, 0, 3, 4, 1).reshape(gn * MB_NODES, 3)
            lo = base + g0 * MB_NODES
            y[lo:lo + gn * MB_NODES, :3] = blk
            off += n
    y = y.reshape(-1)[:N_NODES * 4].copy()
    y.reshape(-1, 4)[:, 3] = vr_full.reshape(-1, 4)[:, 3]
    return y


def kernel(x, weights, VR):
    from concourse import bass_utils

    x = np.asarray(x, dtype=np.float32)
    weights = np.asarray(weights, dtype=np.float32)
    VR = np.asarray(VR, dtype=np.float32)

    wdt = np.float16 if W_DT == "f16" else np.float32
    qm4 = x.reshape(10, 4)
    # blend matrix with planar-by-12 column permutation:
    # column 12*j + s holds component j of dozen-node s
    bd = np.zeros((120, 48), np.float32)
    for s in range(12):
        for j in range(4):
            bd[10 * s:10 * s + 10, 12 * j + s] = qm4[:, j]
    bd = bd.astype(wdt)

    w_pad = np.zeros((N_PAD, 10), np.float32)
    w_pad[:N_NODES] = weights
    vr_pad = np.zeros(N_PAD * 4, np.float32)
    vr_pad[:N_NODES * 4] = VR

    wt_shards = _host_wt(w_pad)
    vr_shards = _host_vr_planar(vr_pad)

    nc = _get_compiled()
    in_maps = [
        {"wt": wt_shards[c], "vr": vr_shards[c], "bd": bd}
        for c in range(N_CORES)
    ]
    res = bass_utils.run_bass_kernel_spmd(nc, in_maps, core_ids=list(range(N_CORES)))
    y_blocks = [res.results[c]["y"] for c in range(N_CORES)]
    return _host_y_reassemble(y_blocks, VR)


if __name__ == "__main__":
    # quick self-check with random data
    rng = np.random.default_rng(0)
    x = rng.standard_normal(40).astype(np.float32)
    W = (rng.standard_normal((N_NODES, 10)) * 0.1).astype(np.float32)
    VR = rng.standard_normal(N_NODES * 4).astype(np.float32)
    y = kernel(x, weights=W, VR=VR)
    print("kernel ran, y shape", y.shape, y[:8])
